# revision 28
# baseline (speedup 1.0000x reference)
"""Trainium2 Bass kernel for nn_DriftingModel (drifting-loss Sinkhorn).

Self-contained: kernel(**inputs) -> np.ndarray [N] float32.

8 NeuronCores, row-sharded data parallel on N. Host->device payload is
minimized (the axon tunnel is the wall-clock bottleneck): each core
receives only its z/pos row shard and a 1/8 row-shard of every MLP
weight; full weights, pos (row-major), posT (via on-device PE
transposes) and sq_pos are reconstructed on device with HBM AllGathers.

gen = MLP(z) on PE in transposed layout; dist [N, 2N] built once via PE
Gram matmuls in both row-major and col-major layouts, stored fp32 in
HBM. Sinkhorn (5 iters) in log domain via the shift recurrence
r_k = rowLSE(L0 - c_{k-1}), c_k = colLSE(L0 - r_k). Row passes stream
row-major dist: fused tensor_scalar(mult -1/T, max-accum) row max + ACT
exp(accum_out) row sums; column sums of exp(L0 - r_k) in the same pass
via fp16 weighted matmuls (w = 1/s) PSUM-packed 4 tiles/bank
(tile_position col groups), AllReduced across cores. c_1 uses a
dedicated col-major pass (exact per-column max, AllGather LSE-combine).
Final pass builds A col-major, P1t/P2t = pos^T A_p^T / gen^T A_n^T on
PE, a_p/a_n via ones-matmuls, loss_i = sum_d V^2 via Square +
ones-matmul.
"""
import sys
import numpy as np

try:
    import concourse.bass as bass
except ImportError:
    sys.path.insert(0, "/opt/trn_rl_repo")
    import concourse.bass as bass
import concourse.bacc as bacc
import concourse.mybir as mybir
import concourse.tile as tile
from concourse import bass_utils

F32 = mybir.dt.float32
F16 = mybir.dt.float16
U32 = mybir.dt.uint32
AF = mybir.ActivationFunctionType
ALU = mybir.AluOpType

TEMP = 0.05
SCL = -1.0 / TEMP
BIG = 1e6
LAM = 1.0507009873554805
ALPHA = 1.6732632423543772
LA = LAM * ALPHA


def build_program(NC, SH, D, ND, H, n_iters=5, wire16=True):
    N = NC * SH
    NJ = 2 * N
    RB = SH // 128
    NT = NJ // 512
    CHW = min(2048, NJ)
    NCH = NJ // CHW
    CPT = CHW // 512
    NBLK = NJ // 128
    HS = H // 128
    DS = D // 128
    IW = min(512, SH)
    ISC = SH // IW
    NBANK = min(8, NT)

    # flat wire layout (one f16 + one f32 array per core; see host_inputs).
    # z and the MLP weights ride fp16 (their rounding shifts the loss by
    # ~7e-3 rel, inside tolerance); pos/biases stay f32.
    ZW = ND * SH
    PW = SH * D
    W1W = (ND // NC) * H
    WHW = (H // NC) * H
    W5W = (H // NC) * D
    WSH_W = W1W + 3 * WHW + W5W
    T16 = ZW + WSH_W
    OZT, OW = 0, ZW
    WOFF = [0, W1W, W1W + WHW, W1W + 2 * WHW, W1W + 3 * WHW]
    LBW = 128 * HS
    OPOS = 0
    OLB = PW
    OEB = OLB + 4 * LBW
    OB5 = OEB + 4 * LBW
    OID = OB5 + 128 * DS
    T32 = OID + 128 * 128

    nc = bacc.Bacc("TRN2", target_bir_lowering=False, debug=False,
                   num_devices=NC)

    def din(name, shape, dt=F32):
        return nc.dram_tensor(name, shape, dt, kind="ExternalInput")

    flat_in = din("flat16", [1, T16], F16 if wire16 else F32)
    flat32 = din("flat32", [1, T32])
    loss = nc.dram_tensor("loss", [1, SH], F32, kind="ExternalOutput")

    with tile.TileContext(nc) as tc:
      with tc.tile_pool(name="glob", bufs=1) as gp, \
           tc.tile_pool(name="psq", bufs=1, space="PSUM") as pq, \
           tc.tile_pool(name="dram", bufs=1, space="DRAM") as dram:
        genT = [gp.tile([128, SH], F32, name=f"genT{i}", tag=f"genT{i}") for i in range(DS)]
        m2genT = [gp.tile([128, SH], F32, name=f"m2genT{i}", tag=f"m2genT{i}") for i in range(DS)]
        sqg_row = gp.tile([1, SH], F32, tag="sqg_row")
        sq_pp = gp.tile([128, NBLK], F32, tag="sq_pp")
        nsq_pp = gp.tile([128, NBLK], F32, tag="nsq_pp")
        sqg_pp = gp.tile([128, RB], F32, tag="sqg_pp")
        nsqg_pp = gp.tile([128, RB], F32, tag="nsqg_pp")
        r_pp = gp.tile([128, RB], F32, tag="r_pp")
        c_pp = gp.tile([128, NBLK], F32, tag="c_pp")
        negc_pp = gp.tile([128, NBLK], F32, tag="negc_pp")
        con1 = gp.tile([1, 128], F32, tag="con1")
        con128 = gp.tile([128, 1], F32, tag="con128")
        idt = gp.tile([128, 128], F32, tag="idt")
        ibt = gp.tile([128, 128], F32, tag="ibt")
        nc.vector.memset(con1[:], 1.0)
        nc.vector.memset(con128[:], 1.0)
        nc.sync.dma_start(idt[:],
                          flat32[0, OID:OID + 128 * 128]
                          .rearrange("(p f) -> p f", p=128))
        nc.vector.tensor_scalar_mul(ibt[:], idt[:], BIG)
        nc.gpsimd.memset(c_pp[:], 0.0)

        dist_hbm = dram.tile([SH, NJ], F32, tag="dist_hbm")
        distT_hbm = dram.tile([NJ, SH], F32, tag="distT_hbm")
        genT_ag_in = dram.tile([D, SH], F32, tag="genT_ag_in")
        genT_ag_out = dram.tile([NC * D, SH], F32, tag="genT_ag_out")
        gen_ag_in = dram.tile([SH, D], F32, tag="gen_ag_in")
        gen_full = dram.tile([N, D], F32, tag="gen_full")
        sqg_ag_in = dram.tile([1, SH], F32, tag="sqg_ag_in")
        sqg_ag_out = dram.tile([NC, SH], F32, tag="sqg_ag_out")
        sq_dram = dram.tile([1, NJ], F32, tag="sq_dram")
        row_dram = dram.tile([1, max(SH, NJ)], F32, tag="row_dram")
        ct_row_dram = dram.tile([1, NJ], F32, tag="ct_row_dram")
        rt_row_dram = dram.tile([1, SH], F32, tag="rt_row_dram")
        ap_dram = dram.tile([1, SH], F32, tag="ap_dram")
        an_dram = dram.tile([1, SH], F32, tag="an_dram")
        cstat_in = dram.tile([2, NJ], F32, tag="cstat_in")
        cstat_out = dram.tile([2 * NC, NJ], F32, tag="cstat_out")
        scol_in = dram.tile([1, NJ], F32, tag="scol_in")
        scol_out = dram.tile([1, NJ], F32, tag="scol_out")
        pos_full = dram.tile([N, D], F32, tag="pos_full")
        posT_ag_in = dram.tile([D, SH], F32, tag="posT_ag_in")
        posT_ag_out = dram.tile([NC * D, SH], F32, tag="posT_ag_out")
        sqp_ag_in = dram.tile([1, SH], F32, tag="sqp_ag_in")
        sqp_ag_out = dram.tile([NC, SH], F32, tag="sqp_ag_out")
        rg = [list(range(NC))]

        # unpack the flat wire payload: f16 -> f32 once through SBUF
        fl = dram.tile([1, T16], F32, tag="fl")
        if wire16:
            with tc.tile_pool(name="cvt", bufs=1) as cv:
                c16 = cv.tile([128, T16 // 128], F16, tag="c16")
                nc.sync.dma_start(c16[:],
                                  flat_in[0, :]
                                  .rearrange("(p f) -> p f", p=128))
                c32 = cv.tile([128, T16 // 128], F32, tag="c32")
                nc.vector.tensor_copy(c32[:], c16[:])
                nc.sync.dma_start(fl[0, :].rearrange("(p f) -> p f", p=128),
                                  c32[:])
        else:
            nc.sync.dma_start(fl[:, :], flat_in[:, :])

        # gather sharded weights + pos (cuts host->device payload 8x);
        # collectives can't read IO tensors, so shards are staged via fl
        wst_all = dram.tile([1, WSH_W], F32, tag="wst_all")
        wall = dram.tile([NC, WSH_W], F32, tag="wall")
        pos_st = dram.tile([SH, D], F32, tag="pos_st")
        nc.sync.dma_start(wst_all[0, :], fl[0, OW:OW + WSH_W])
        nc.gpsimd.collective_compute("AllGather", ALU.bypass,
                                     ins=[wst_all.opt()],
                                     outs=[wall.opt()],
                                     replica_groups=rg)
        nc.sync.dma_start(pos_st[:, :],
                          flat32[0, OPOS:OPOS + PW]
                          .rearrange("(s d) -> s d", s=SH))
        nc.gpsimd.collective_compute("AllGather", ALU.bypass,
                                     ins=[pos_st.opt()],
                                     outs=[pos_full.opt()],
                                     replica_groups=rg)

        # ================= Phase 0: MLP (transposed) =================
        with tc.tile_pool(name="mlp_w", bufs=1) as wp, \
             tc.tile_pool(name="mlp_h", bufs=1) as hp, \
             tc.tile_pool(name="mlp_t", bufs=3) as tp:
            hTa = [hp.tile([128, SH], F32, name=f"hTa{s}", tag=f"hTa{s}") for s in range(HS)]
            hTb = [hp.tile([128, SH], F32, name=f"hTb{s}", tag=f"hTb{s}") for s in range(HS)]

            def selu_slice(ps, lb, eb, s, dst):
                pt = tp.tile([128, SH], F32, tag="selu_p")
                nc.scalar.activation(pt[:], ps[:], AF.Relu,
                                     bias=lb[:, s:s+1], scale=LAM)
                et = tp.tile([128, SH], F32, tag="selu_e")
                nc.scalar.activation(et[:], ps[:], AF.Exp,
                                     bias=eb[:, s:s+1], scale=1.0)
                nc.vector.tensor_scalar(out=et[:], in0=et[:], scalar1=LA,
                                        scalar2=None, op0=ALU.min)
                nc.vector.tensor_add(dst[:], pt[:], et[:])

            # pos shard -> posT via PE transpose + sq_pos shard, to AllGather
            pT = [hp.tile([128, SH], F32, name=f"pT{db}", tag=f"pT{db}")
                  for db in range(DS)]
            for ib in range(RB):
                psh = tp.tile([128, D], F32, tag="gsh")
                nc.sync.dma_start(
                    psh[:],
                    flat32[0, OPOS + ib*128*D:OPOS + (ib+1)*128*D]
                    .rearrange("(p d) -> p d", p=128))
                for db in range(DS):
                    tps = pq.tile([128, 128], F32, name="ptr_ps", tag="w3")
                    nc.tensor.transpose(tps[:], psh[:, db*128:(db+1)*128],
                                        idt[:])
                    nc.vector.tensor_copy(pT[db][:, ib*128:(ib+1)*128],
                                          tps[:])
            for db in range(DS):
                nc.sync.dma_start(posT_ag_in[db*128:(db+1)*128, :], pT[db][:])
            sqp_big = pq.tile([128, SH], F32, tag="w2")
            sqp_ps = sqp_big[0:1, :]
            for db in range(DS):
                sqt = tp.tile([128, SH], F32, tag="selu_e")
                nc.scalar.activation(sqt[:], pT[db][:], AF.Square)
                for ic in range(ISC):
                    nc.tensor.matmul(sqp_ps[:, ic*IW:(ic+1)*IW],
                                     con128[:, 0:1], sqt[:, ic*IW:(ic+1)*IW],
                                     start=(db == 0), stop=(db == DS-1))
            sqp_row = tp.tile([1, SH], F32, tag="sqp_row")
            nc.vector.tensor_copy(sqp_row[:], sqp_ps[:])
            nc.sync.dma_start(sqp_ag_in[:], sqp_row[:])
            nc.gpsimd.collective_compute("AllGather", ALU.bypass,
                                         ins=[posT_ag_in.opt()],
                                         outs=[posT_ag_out.opt()],
                                         replica_groups=rg)
            nc.gpsimd.collective_compute("AllGather", ALU.bypass,
                                         ins=[sqp_ag_in.opt()],
                                         outs=[sqp_ag_out.opt()],
                                         replica_groups=rg)

            # layer 1 (K = ND = 128)
            w1 = wp.tile([ND, H], F32, tag="w_first")
            for c8 in range(NC):
                nc.sync.dma_start(
                    w1[c8*(ND//NC):(c8+1)*(ND//NC), :],
                    wall[c8, WOFF[0]:WOFF[0] + W1W]
                    .rearrange("(p f) -> p f", p=ND // NC))
            zT_sb = wp.tile([ND, SH], F32, tag="zT_sb")
            nc.sync.dma_start(zT_sb[:],
                              fl[0, OZT:OZT + ZW]
                              .rearrange("(p f) -> p f", p=ND))
            lb = wp.tile([128, HS], F32, tag="lb")
            nc.sync.dma_start(lb[:],
                              flat32[0, OLB:OLB + LBW]
                              .rearrange("(p f) -> p f", p=128))
            eb = wp.tile([128, HS], F32, tag="eb")
            nc.sync.dma_start(eb[:],
                              flat32[0, OEB:OEB + LBW]
                              .rearrange("(p f) -> p f", p=128))
            for s in range(HS):
                ps = pq.tile([128, SH], F32, name=f"l1ps{s}", tag=f"w{s % 4}")
                for ic in range(ISC):
                    nc.tensor.matmul(ps[:, ic*IW:(ic+1)*IW],
                                     w1[:, s*128:(s+1)*128],
                                     zT_sb[:, ic*IW:(ic+1)*IW],
                                     start=True, stop=True)
                selu_slice(ps, lb, eb, s, hTa[s])
            hT, hT2 = hTa, hTb
            # layers 2..4 (K = H)
            for l in range(1, 4):
                wl = [wp.tile([128, H], F32, name=f"w_kb{kb}", tag=f"w_kb{kb}")
                      for kb in range(HS)]
                for kb in range(HS):
                    nc.sync.dma_start(wl[kb][:],
                                      wall[kb, WOFF[l]:WOFF[l] + WHW]
                                      .rearrange("(p f) -> p f", p=128))
                lb = wp.tile([128, HS], F32, tag="lb")
                nc.sync.dma_start(lb[:],
                                  flat32[0, OLB + l*LBW:OLB + (l+1)*LBW]
                                  .rearrange("(p f) -> p f", p=128))
                eb = wp.tile([128, HS], F32, tag="eb")
                nc.sync.dma_start(eb[:],
                                  flat32[0, OEB + l*LBW:OEB + (l+1)*LBW]
                                  .rearrange("(p f) -> p f", p=128))
                for s in range(HS):
                    ps = pq.tile([128, SH], F32, name=f"l{l}ps{s}",
                                 tag=f"w{s % 4}")
                    for ic in range(ISC):
                        for kb in range(HS):
                            nc.tensor.matmul(
                                ps[:, ic*IW:(ic+1)*IW],
                                wl[kb][:, s*128:(s+1)*128],
                                hT[kb][:, ic*IW:(ic+1)*IW],
                                start=(kb == 0), stop=(kb == HS-1))
                    selu_slice(ps, lb, eb, s, hT2[s])
                hT, hT2 = hT2, hT
            # layer 5 -> genT
            w5 = [wp.tile([128, D], F32, name=f"w5_kb{kb}", tag=f"w5_kb{kb}")
                  for kb in range(HS)]
            for kb in range(HS):
                nc.sync.dma_start(w5[kb][:],
                                  wall[kb, WOFF[4]:WOFF[4] + W5W]
                                  .rearrange("(p f) -> p f", p=128))
            b5 = wp.tile([128, DS], F32, tag="b5")
            nc.sync.dma_start(b5[:],
                              flat32[0, OB5:OB5 + 128 * DS]
                              .rearrange("(p f) -> p f", p=128))
            for s in range(DS):
                ps = pq.tile([128, SH], F32, name=f"l5ps{s}", tag=f"w{s % 4}")
                for ic in range(ISC):
                    for kb in range(HS):
                        nc.tensor.matmul(
                            ps[:, ic*IW:(ic+1)*IW],
                            w5[kb][:, s*128:(s+1)*128],
                            hT[kb][:, ic*IW:(ic+1)*IW],
                            start=(kb == 0), stop=(kb == HS-1))
                nc.scalar.activation(genT[s][:], ps[:], AF.Identity,
                                     bias=b5[:, s:s+1], scale=1.0)
            nc.vector.tensor_scalar_mul(m2genT[0][:], genT[0][:], -2.0)
            nc.vector.tensor_scalar_mul(m2genT[1][:], genT[1][:], -2.0)

            # sq_gen shard
            sq_big = pq.tile([128, SH], F32, tag="w2")
            sq_ps = sq_big[0:1, :]
            for db in range(DS):
                sqt = tp.tile([128, SH], F32, tag="selu_p")
                nc.scalar.activation(sqt[:], genT[db][:], AF.Square)
                for ic in range(ISC):
                    nc.tensor.matmul(sq_ps[:, ic*IW:(ic+1)*IW],
                                     con128[:, 0:1], sqt[:, ic*IW:(ic+1)*IW],
                                     start=(db == 0), stop=(db == DS-1))
            nc.vector.tensor_copy(sqg_row[:], sq_ps[:])
            nc.sync.dma_start(sqg_ag_in[:], sqg_row[:])

            # transpose gen shard -> gen rows layout, send to AG
            for ib in range(RB):
                gsh = tp.tile([128, D], F32, tag="gsh")
                for db in range(DS):
                    tps = pq.tile([128, 128], F32, name="tr_ps", tag="w3")
                    nc.tensor.transpose(tps[:],
                                        genT[db][:, ib*128:(ib+1)*128],
                                        idt[:])
                    nc.vector.tensor_copy(gsh[:, db*128:(db+1)*128], tps[:])
                nc.sync.dma_start(gen_ag_in[ib*128:(ib+1)*128, :], gsh[:])
            for db in range(DS):
                nc.sync.dma_start(genT_ag_in[db*128:(db+1)*128, :],
                                  genT[db][:])

        ag1 = nc.gpsimd.collective_compute("AllGather", ALU.bypass,
                                           ins=[gen_ag_in.opt()],
                                           outs=[gen_full.opt()],
                                           replica_groups=rg)
        nc.gpsimd.collective_compute("AllGather", ALU.bypass,
                                     ins=[genT_ag_in.opt()],
                                     outs=[genT_ag_out.opt()],
                                     replica_groups=rg)
        ag3 = nc.gpsimd.collective_compute("AllGather", ALU.bypass,
                                           ins=[sqg_ag_in.opt()],
                                           outs=[sqg_ag_out.opt()],
                                           replica_groups=rg)

        nc.sync.dma_start(sq_dram[0, 0:N],
                          sqp_ag_out[:, :].rearrange("c s -> (c s)"))
        nc.sync.dma_start(sq_dram[0, N:NJ],
                          sqg_ag_out[:, :].rearrange("c s -> (c s)"))
        nc.sync.dma_start(sq_pp[:],
                          sq_dram[0, :].rearrange("(b p) -> p b", p=128))
        nc.vector.tensor_scalar_mul(nsq_pp[:], sq_pp[:], -1.0)
        nc.sync.dma_start(sqg_pp[:],
                          sqg_ag_in[0, :].rearrange("(b p) -> p b", p=128))
        nc.vector.tensor_scalar_mul(nsqg_pp[:], sqg_pp[:], -1.0)

        # ============ Phase 1: build dist (both layouts) ============
        with tc.tile_pool(name="yt", bufs=1) as yp, \
             tc.tile_pool(name="bld", bufs=3) as bp:
            ytop = yp.tile([128, NJ], F32, tag="ytop")
            ybot = yp.tile([128, NJ], F32, tag="ybot")
            for c in range(NC):
                nc.sync.dma_start(ytop[:, c*SH:(c+1)*SH],
                                  posT_ag_out[c*D:c*D+128, :])
                nc.sync.dma_start(ybot[:, c*SH:(c+1)*SH],
                                  posT_ag_out[c*D+128:c*D+256, :])
            for c in range(NC):
                nc.sync.dma_start(ytop[:, N+c*SH:N+(c+1)*SH],
                                  genT_ag_out[c*D:c*D+128, :])
                nc.sync.dma_start(ybot[:, N+c*SH:N+(c+1)*SH],
                                  genT_ag_out[c*D+128:c*D+256, :])
            for b in range(RB):
                for t in range(NT):
                    ps = pq.tile([128, 512], F32, name=f"d2ps{t % 4}",
                                 tag=f"w{t % 4}")
                    nc.tensor.matmul(ps[:],
                                     m2genT[0][:, b*128:(b+1)*128],
                                     ytop[:, t*512:(t+1)*512],
                                     start=True, stop=False)
                    nc.tensor.matmul(ps[:], m2genT[1][:, b*128:(b+1)*128],
                                     ybot[:, t*512:(t+1)*512],
                                     start=False, stop=False)
                    sqs = bp.tile([1, 512], F32, tag="sqs")
                    nc.sync.dma_start(sqs[:],
                                      sq_dram[0:1, t*512:(t+1)*512])
                    nc.tensor.matmul(ps[:], con1[0:1, :], sqs[0:1, :],
                                     start=False, stop=True)
                    dd = bp.tile([128, 512], F32, tag="dd_row")
                    nc.vector.tensor_scalar(out=dd[:], in0=ps[:],
                                            scalar1=nsqg_pp[:, b:b+1],
                                            scalar2=None, op0=ALU.max)
                    dt_ = bp.tile([128, 512], F32, tag="dist_row")
                    nc.scalar.activation(dt_[:], dd[:], AF.Sqrt,
                                         bias=sqg_pp[:, b:b+1])
                    nc.sync.dma_start(dist_hbm[b*128:(b+1)*128,
                                               t*512:(t+1)*512], dt_[:])
            for jb in range(NBLK):
                ps = pq.tile([128, SH], F32, name=f"d2T{jb % 4}",
                             tag=f"w{jb % 4}")
                for ic in range(ISC):
                    nc.tensor.matmul(ps[:, ic*IW:(ic+1)*IW],
                                     ytop[:, jb*128:(jb+1)*128],
                                     m2genT[0][:, ic*IW:(ic+1)*IW],
                                     start=True, stop=False)
                    nc.tensor.matmul(ps[:, ic*IW:(ic+1)*IW],
                                     ybot[:, jb*128:(jb+1)*128],
                                     m2genT[1][:, ic*IW:(ic+1)*IW],
                                     start=False, stop=False)
                    nc.tensor.matmul(ps[:, ic*IW:(ic+1)*IW], con1[0:1, :],
                                     sqg_row[0:1, ic*IW:(ic+1)*IW],
                                     start=False, stop=True)
                dd = bp.tile([128, SH], F32, tag="dd_col")
                colbuild_last = nc.vector.tensor_scalar(
                    out=dd[:], in0=ps[:], scalar1=nsq_pp[:, jb:jb+1],
                    scalar2=None, op0=ALU.max)
                dt_ = bp.tile([128, SH], F32, tag="dist_col")
                nc.scalar.activation(dt_[:], dd[:], AF.Sqrt,
                                     bias=sq_pp[:, jb:jb+1])
                nc.sync.dma_start(distT_hbm[jb*128:(jb+1)*128, :], dt_[:])

        # diag patches (+BIG on masked diagonal), dynamic col/row offsets
        with tc.tile_pool(name="patch", bufs=2) as pb:
            pid = nc.gpsimd.partition_id()
            reg = nc.gpsimd.alloc_register("doff")
            nc.gpsimd.reg_mul(reg, pid, SH)
            nc.gpsimd.reg_add(reg, reg, N)
            doff = nc.gpsimd.snap(reg, min_val=N, max_val=NJ - SH)
            for b in range(RB):
                pt = pb.tile([128, 128], F32, tag="ptile")
                nc.gpsimd.dma_start(
                    pt[:], dist_hbm[b*128:(b+1)*128,
                                    bass.DynSlice(doff + b*128, 128)])
                pt2 = pb.tile([128, 128], F32, tag="ptile2")
                nc.vector.tensor_add(pt2[:], pt[:], ibt[:])
                nc.gpsimd.dma_start(
                    dist_hbm[b*128:(b+1)*128,
                             bass.DynSlice(doff + b*128, 128)], pt2[:])
            for b in range(RB):
                pt = pb.tile([128, 128], F32, tag="ptile")
                nc.gpsimd.dma_start(
                    pt[:], distT_hbm[bass.DynSlice(doff + b*128, 128),
                                     b*128:(b+1)*128])
                pt2 = pb.tile([128, 128], F32, tag="ptile2")
                nc.vector.tensor_add(pt2[:], pt[:], ibt[:])
                nc.gpsimd.dma_start(
                    distT_hbm[bass.DynSlice(doff + b*128, 128),
                              b*128:(b+1)*128], pt2[:])

        def make_rt_row(sp_):
            rt_pp = sp_.tile([128, RB], F32, tag="rt_pp")
            nc.vector.tensor_scalar_mul(rt_pp[:], r_pp[:], TEMP)
            nc.sync.dma_start(
                rt_row_dram[0, :].rearrange("(b p) -> p b", p=128), rt_pp[:])

        def row_pass(k):
            with tc.tile_pool(name=f"rq{k}", bufs=NCH + 1) as qp, \
                 tc.tile_pool(name=f"re{k}", bufs=NCH + 1) as ep, \
                 tc.tile_pool(name=f"rs{k}", bufs=3) as sp_:
                cbank = None
                if k > 1:
                    nbt = (NBANK + 1) // 2
                    cbt_ = [pq.tile([128, 1024], F32, name=f"cbk{k}_{i}",
                                    tag=f"w{i}") for i in range(nbt)]
                    for t_ in cbt_:
                        nc.vector.memset(t_[:], 0.0)
                    cbank = [cbt_[i // 2][:, (i % 2)*512:(i % 2)*512+512]
                             for i in range(NBANK)]
                for b in range(RB):
                    mpart = sp_.tile([128, NCH], F32, tag="mpart")
                    spart = sp_.tile([128, NCH], F32, tag="spart")
                    qs = []
                    for ch in range(NCH):
                        q = qp.tile([128, CHW], F32, tag="q")
                        if k == 1:
                            nc.sync.dma_start(
                                q[:], dist_hbm[b*128:(b+1)*128,
                                               ch*CHW:(ch+1)*CHW])
                        else:
                            nc.sync.dma_start(
                                q[:],
                                ct_row_dram[0, ch*CHW:(ch+1)*CHW]
                                .partition_broadcast(128))
                            nc.gpsimd.dma_start(
                                q[:], dist_hbm[b*128:(b+1)*128,
                                               ch*CHW:(ch+1)*CHW],
                                accum_op=ALU.add)
                        nc.vector.tensor_scalar(
                            out=q[:], in0=q[:], scalar1=SCL, scalar2=None,
                            op0=ALU.mult, op1=ALU.max,
                            accum_out=mpart[:, ch:ch+1])
                        qs.append(q)
                    mb = sp_.tile([128, 1], F32, tag="mb")
                    nc.vector.tensor_reduce(out=mb[:], in_=mpart[:],
                                            op=ALU.max,
                                            axis=mybir.AxisListType.X)
                    nmb = sp_.tile([128, 1], F32, tag="nmb")
                    nc.vector.tensor_scalar_mul(nmb[:], mb[:], -1.0)
                    es = []
                    for ch in range(NCH):
                        e = ep.tile([128, CHW], F16, tag="e")
                        nc.scalar.activation(e[:], qs[ch][:], AF.Exp,
                                             bias=nmb[:, 0:1], scale=1.0,
                                             accum_out=spart[:, ch:ch+1])
                        es.append(e)
                    sb_ = sp_.tile([128, 1], F32, tag="sb_")
                    nc.vector.tensor_reduce(out=sb_[:], in_=spart[:],
                                            op=ALU.add,
                                            axis=mybir.AxisListType.X)
                    lnsb = sp_.tile([128, 1], F32, tag="lnsb")
                    nc.scalar.activation(lnsb[:], sb_[:], AF.Ln)
                    nc.vector.tensor_add(r_pp[:, b:b+1], mb[:], lnsb[:])
                    if k > 1:
                        w32 = sp_.tile([128, 1], F32, tag="w32")
                        nc.vector.reciprocal(w32[:], sb_[:])
                        w16 = sp_.tile([128, 1], F16, tag="w16")
                        nc.vector.tensor_copy(w16[:], w32[:])
                        for ch in range(NCH):
                            for n in range(CPT):
                                t = ch * CPT + n
                                bank, grp = t % NBANK, t // NBANK
                                nc.tensor.matmul(
                                    cbank[bank][32*grp:32*grp+1, :],
                                    w16[:, 0:1], es[ch][:, n*512:(n+1)*512],
                                    start=(b == 0), stop=(b == RB-1),
                                    tile_position=(0, 32*grp))
                if k > 1:
                    for bank in range(NBANK):
                        sc = sp_.tile([97, 512], F32, tag="scol")
                        nc.vector.tensor_copy(sc[:], cbank[bank][0:97, :])
                        for grp in range(NT // NBANK):
                            t = grp * NBANK + bank
                            nc.sync.dma_start(
                                scol_in[0:1, t*512:(t+1)*512],
                                sc[32*grp:32*grp+1, :])
                    nc.gpsimd.collective_compute(
                        "AllReduce", ALU.add, ins=[scol_in.opt()],
                        outs=[scol_out.opt()], replica_groups=rg)
                    spp = sp_.tile([128, NBLK], F32, tag="spp")
                    nc.sync.dma_start(
                        spp[:],
                        scol_out[0, :].rearrange("(b p) -> p b", p=128))
                    lns = sp_.tile([128, NBLK], F32, tag="lns")
                    nc.scalar.activation(lns[:], spp[:], AF.Ln)
                    last = nc.vector.tensor_add(c_pp[:], c_pp[:], lns[:])
                    return last
            return None

        # ================= Phase 2: R1 =================
        row_pass(1)

        # ========== Phase 3: c1 stats (col-major, exact) ==========
        with tc.tile_pool(name="c1", bufs=6) as cp, \
             tc.tile_pool(name="c1s", bufs=2) as csp:
            make_rt_row(csp)
            mstat = csp.tile([128, NBLK], F32, tag="mstat")
            sstat = csp.tile([128, NBLK], F32, tag="sstat")
            for jb in range(NBLK):
                q = cp.tile([128, SH], F32, tag="c1q")
                nc.sync.dma_start(
                    q[:], rt_row_dram[0, :].partition_broadcast(128))
                nc.gpsimd.dma_start(q[:], distT_hbm[jb*128:(jb+1)*128, :],
                                    accum_op=ALU.add)
                nc.vector.tensor_scalar(
                    out=q[:], in0=q[:], scalar1=SCL, scalar2=None,
                    op0=ALU.mult, op1=ALU.max, accum_out=mstat[:, jb:jb+1])
                nmj = cp.tile([128, 1], F32, tag="nmj")
                nc.vector.tensor_scalar_mul(nmj[:], mstat[:, jb:jb+1], -1.0)
                ed = cp.tile([128, SH], F32, tag="c1e")
                nc.scalar.activation(ed[:], q[:], AF.Exp, bias=nmj[:, 0:1],
                                     scale=1.0, accum_out=sstat[:, jb:jb+1])
            nc.sync.dma_start(
                cstat_in[0, :].rearrange("(b p) -> p b", p=128), mstat[:])
            nc.sync.dma_start(
                cstat_in[1, :].rearrange("(b p) -> p b", p=128), sstat[:])
            nc.gpsimd.collective_compute(
                "AllGather", ALU.bypass, ins=[cstat_in.opt()],
                outs=[cstat_out.opt()], replica_groups=rg)
            mc, sc_ = [], []
            for c in range(NC):
                m_ = csp.tile([128, NBLK], F32, tag=f"mc{c}")
                nc.sync.dma_start(
                    m_[:], cstat_out[2*c, :].rearrange("(b p) -> p b", p=128))
                s_ = csp.tile([128, NBLK], F32, tag=f"sc{c}")
                nc.sync.dma_start(
                    s_[:],
                    cstat_out[2*c+1, :].rearrange("(b p) -> p b", p=128))
                mc.append(m_)
                sc_.append(s_)
            mg = csp.tile([128, NBLK], F32, tag="mg")
            nc.vector.tensor_max(mg[:], mc[0][:], mc[1][:])
            for c in range(2, NC):
                nc.vector.tensor_max(mg[:], mg[:], mc[c][:])
            acc = csp.tile([128, NBLK], F32, tag="acc")
            nc.gpsimd.memset(acc[:], 0.0)
            for c in range(NC):
                dm = csp.tile([128, NBLK], F32, tag="dm")
                nc.vector.tensor_sub(dm[:], mc[c][:], mg[:])
                edm = csp.tile([128, NBLK], F32, tag="edm")
                nc.scalar.activation(edm[:], dm[:], AF.Exp)
                nc.vector.tensor_mul(edm[:], edm[:], sc_[c][:])
                nc.vector.tensor_add(acc[:], acc[:], edm[:])
            lacc = csp.tile([128, NBLK], F32, tag="lacc")
            nc.scalar.activation(lacc[:], acc[:], AF.Ln)
            nc.vector.tensor_add(c_pp[:], mg[:], lacc[:])

        def make_ct_row(hp_):
            ct_pp = hp_.tile([128, NBLK], F32, tag="ct_pp")
            nc.vector.tensor_scalar_mul(ct_pp[:], c_pp[:], TEMP)
            nc.sync.dma_start(
                ct_row_dram[0, :].rearrange("(b p) -> p b", p=128), ct_pp[:])

        # ================= Phases 4..7: R2..R5 =================
        for k in range(2, n_iters + 1):
            with tc.tile_pool(name=f"cbh{k}", bufs=2) as hp_:
                make_ct_row(hp_)
            row_pass(k)

        # ================= Phase 8: final =================
        NG = SH // IW
        with tc.tile_pool(name="fin", bufs=4) as fp_, \
             tc.tile_pool(name="fins", bufs=2) as fsp:
            nc.vector.tensor_scalar_mul(negc_pp[:], c_pp[:], -1.0)
            make_rt_row(fsp)
            p1t = [None, None]
            p2t = [None, None]
            for half in range(2):
                pps = [pq.tile([128, SH], F32, name=f"pps{half}_{db}",
                               tag=f"w{db}") for db in range(DS)]
                aps = pq.tile([128, 512], F32, name=f"aps{half}", tag="w2")
                nc.vector.memset(aps[:], 0.0)
                for j0 in range(NBLK // 2):
                    jb = half * (NBLK // 2) + j0
                    q = fp_.tile([128, SH], F32, tag="fq")
                    nc.sync.dma_start(
                        q[:], rt_row_dram[0, :].partition_broadcast(128))
                    nc.gpsimd.dma_start(q[:],
                                        distT_hbm[jb*128:(jb+1)*128, :],
                                        accum_op=ALU.add)
                    a = fp_.tile([128, SH], F32, tag="fa")
                    nc.scalar.activation(a[:], q[:], AF.Exp,
                                         bias=negc_pp[:, jb:jb+1], scale=SCL)
                    pg = fp_.tile([128, D], F32, tag="fpg")
                    if half == 0:
                        nc.sync.dma_start(pg[:],
                                          pos_full[jb*128:(jb+1)*128, :])
                    else:
                        nc.sync.dma_start(pg[:],
                                          gen_full[j0*128:(j0+1)*128, :])
                    for db in range(DS):
                        for ic in range(ISC):
                            nc.tensor.matmul(
                                pps[db][:, ic*IW:(ic+1)*IW],
                                pg[:, db*128:(db+1)*128],
                                a[:, ic*IW:(ic+1)*IW],
                                start=(j0 == 0), stop=(j0 == NBLK//2 - 1))
                    for g in range(NG):
                        nc.tensor.matmul(
                            aps[32*g:32*g+1, 0:IW], con128[:, 0:1],
                            a[:, g*IW:(g+1)*IW],
                            start=(j0 == 0), stop=(j0 == NBLK//2 - 1),
                            tile_position=(0, 32*g))
                pt_ = [fsp.tile([128, SH], F32, name=f"P{half}d{db}", tag=f"P{half}d{db}")
                       for db in range(DS)]
                for db in range(DS):
                    nc.vector.tensor_copy(pt_[db][:], pps[db][:])
                if half == 0:
                    p1t = pt_
                else:
                    p2t = pt_
                asc = fsp.tile([97, 512], F32, tag=f"asc{half}")
                nc.vector.tensor_copy(asc[:], aps[0:97, :])
                adram = ap_dram if half == 0 else an_dram
                for g in range(NG):
                    nc.sync.dma_start(adram[0:1, g*IW:(g+1)*IW],
                                      asc[32*g:32*g+1, 0:IW])
            ab = [None, None]
            for half in range(2):
                abt = fsp.tile([128, SH], F32, tag=f"ab{half}")
                adram = ap_dram if half == 0 else an_dram
                nc.sync.dma_start(abt[:],
                                  adram[0, :].partition_broadcast(128))
                ab[half] = abt
            lps = pq.tile([128, 512], F32, name="loss_ps", tag="w3")
            nc.vector.memset(lps[:], 0.0)
            for db in range(DS):
                v1 = fsp.tile([128, SH], F32, tag="v1")
                nc.vector.tensor_mul(v1[:], p1t[db][:], ab[1][:])
                v2 = fsp.tile([128, SH], F32, tag="v2")
                nc.vector.tensor_mul(v2[:], p2t[db][:], ab[0][:])
                nc.vector.tensor_sub(v1[:], v1[:], v2[:])
                sq = fsp.tile([128, SH], F32, tag="vsq")
                nc.scalar.activation(sq[:], v1[:], AF.Square)
                for g in range(NG):
                    nc.tensor.matmul(lps[32*g:32*g+1, 0:IW], con128[:, 0:1],
                                     sq[:, g*IW:(g+1)*IW],
                                     start=(db == 0), stop=(db == DS-1),
                                     tile_position=(0, 32*g))
            lsc = fsp.tile([97, 512], F32, tag="lsc")
            nc.vector.tensor_copy(lsc[:], lps[0:97, :])
            for g in range(NG):
                nc.sync.dma_start(loss[0:1, g*IW:(g+1)*IW],
                                  lsc[32*g:32*g+1, 0:IW])

    nc.compile()
    return nc


def host_inputs(inputs, NC, SH, D, ND, H, wire16=True):
    f32 = np.float32
    KSH, HSH = ND // NC, H // NC
    pos = np.ascontiguousarray(inputs["pos"], f32)
    z = np.ascontiguousarray(inputs["z"], f32)
    Ws = [np.ascontiguousarray(inputs[f"W{l+1}"], f32) for l in range(5)]
    bs = [np.ascontiguousarray(inputs[f"b{l+1}"], f32) for l in range(5)]
    b_adj = [bs[0]]
    for l in range(1, 5):
        b_adj.append((bs[l].astype(np.float64)
                      - LA * Ws[l].astype(np.float64).sum(axis=0))
                     .astype(f32))
    lb = [np.ascontiguousarray((f32(LAM) * b_adj[l]).reshape(-1, 128).T)
          for l in range(4)]
    eb = [np.ascontiguousarray(
            (b_adj[l] + f32(np.log(LA))).reshape(-1, 128).T)
          for l in range(4)]
    b5pp = np.ascontiguousarray(b_adj[4].reshape(-1, 128).T)
    ident = np.eye(128, dtype=f32)
    f32_tail = np.concatenate(
        [a.ravel() for a in lb] + [a.ravel() for a in eb]
        + [b5pp.ravel(), ident.ravel()])
    wdt = np.float16 if wire16 else f32
    maps = []
    for c in range(NC):
        parts = [z[c*SH:(c+1)*SH, :].T, Ws[0][c*KSH:(c+1)*KSH]]
        parts += [Ws[l][c*HSH:(c+1)*HSH] for l in range(1, 5)]
        f16_flat = np.concatenate([np.asarray(p).ravel() for p in parts])
        f32_flat = np.concatenate([pos[c*SH:(c+1)*SH].ravel(), f32_tail])
        maps.append({"flat16": f16_flat.astype(wdt)[None, :],
                     "flat32": f32_flat[None, :]})
    return maps


_PROG_CACHE = {}
_RUN_CACHE = {}


def _make_runner(nc_b, NC):
    """Per-program dispatch closure with a REUSED jax.jit object.

    run_bass_kernel_spmd builds a fresh jax.jit per call, which costs ~1s
    of retrace/lowering. This replicates its exact dispatch path (same
    _bass_exec_p custom call, same donation) but keeps the jit alive so
    repeat calls hit the C++ pjit fast path.
    """
    import jax
    from jax.sharding import Mesh, PartitionSpec
    from jax.experimental.shard_map import shard_map
    from concourse import bass2jax
    bass2jax.install_neuronx_cc_hook()
    pname = (nc_b.partition_id_tensor.name
             if nc_b.partition_id_tensor else None)
    in_names, out_names, out_avals, zero_shapes = [], [], [], []
    for alloc in nc_b.m.functions[0].allocations:
        if not isinstance(alloc, mybir.MemoryLocationSet):
            continue
        nm = alloc.memorylocations[0].name
        if alloc.kind == "ExternalInput":
            if nm != pname:
                in_names.append(nm)
        elif alloc.kind == "ExternalOutput":
            shape = tuple(alloc.tensor_shape)
            dt_ = mybir.dt.np(alloc.dtype)
            out_names.append(nm)
            out_avals.append(jax.core.ShapedArray(shape, dt_))
            zero_shapes.append((shape, dt_))
    n_params = len(in_names)
    n_outs = len(out_names)
    in_names_full = in_names + out_names + ([pname] if pname else [])

    def _body(*args):
        operands = list(args)
        if pname is not None:
            operands.append(bass2jax.partition_id_tensor())
        return tuple(bass2jax._bass_exec_p.bind(
            *operands, out_avals=tuple(out_avals),
            in_names=tuple(in_names_full), out_names=tuple(out_names),
            lowering_input_output_aliases=(), sim_require_finite=True,
            sim_require_nnan=True, nc=nc_b))

    devices = jax.devices()[:NC]
    mesh = Mesh(np.asarray(devices), ("core",))
    in_specs = (PartitionSpec("core"),) * (n_params + n_outs)
    out_specs = (PartitionSpec("core"),) * n_outs
    donate = tuple(range(n_params, n_params + n_outs))
    sharded = jax.jit(shard_map(_body, mesh=mesh, in_specs=in_specs,
                                out_specs=out_specs, check_rep=False),
                      donate_argnums=donate, keep_unused=True)

    def run(maps):
        concat_in = [np.concatenate([np.asarray(m[nm]) for m in maps],
                                    axis=0) for nm in in_names]
        concat_zeros = [np.zeros((NC * s[0], *s[1:]), d)
                        for s, d in zero_shapes]
        outs = sharded(*concat_in, *concat_zeros)
        outs_np = [np.asarray(o) for o in outs]
        return [{nm: outs_np[i].reshape(NC, *out_avals[i].shape)[c]
                 for i, nm in enumerate(out_names)} for c in range(NC)]

    return run


def kernel(**inputs):
    NC, D, ND, H = 8, 256, 128, 1024
    N = inputs["pos"].shape[0]
    SH = N // NC
    key = (NC, SH, D, ND, H)
    maps = host_inputs(inputs, NC, SH, D, ND, H)
    if key not in _PROG_CACHE:
        nc = build_program(NC, SH, D, ND, H)
        _PROG_CACHE[key] = nc
        res = bass_utils.run_bass_kernel_spmd(nc, maps,
                                              core_ids=list(range(NC)))
        out = np.concatenate([r["loss"][0] for r in res.results])
        try:
            run = _make_runner(nc, NC)
            res2 = run(maps)
            out2 = np.concatenate([r["loss"][0] for r in res2])
            if np.allclose(out, out2, rtol=1e-4, atol=1e-4):
                _RUN_CACHE[key] = run
        except Exception:
            pass
        return out.astype(np.float32)
    run = _RUN_CACHE.get(key)
    if run is None:
        res = bass_utils.run_bass_kernel_spmd(_PROG_CACHE[key], maps,
                                              core_ids=list(range(NC)))
        return np.concatenate(
            [r["loss"][0] for r in res.results]).astype(np.float32)
    return np.concatenate(
        [r["loss"][0] for r in run(maps)]).astype(np.float32)

